# revision 15
# baseline (speedup 1.0000x reference)
"""Trainium2 Bass kernel: BiLSTM classifier (nn_BiLSTMClassifier_11063835755286).

Strategy (8 NeuronCores, pure data-parallel SPMD, no collectives):
  - core k owns batch rows [32k, 32k+32) and runs TWO independent LSTM chains
    (forward tokens + time-flipped tokens), pipelined against each other so
    PE / ACT / DVE overlap across the 512 sequential steps.
  - layout: batch on partitions, gates on the free dim ("form A").
    z_t = [x_t, h_{t-1}] @ [Wi; Wh]  via a 3-chunk augmented-K accumulation
    group in PSUM (no separate input-projection pass, no xp add).
  - embedding rows are fetched with dma_gather (rows -> partitions) and
    transposed to E-on-partitions with PE transposes, 4 steps per tile,
    prefetched ahead of the recurrence.
  - gate order is host-permuted to (i, f, o, g) so one sigmoid covers
    [0:768) and the cell update runs as two fused tensor_tensor ops.
  - final feature transpose + tiny dense (y = [c_fwd|c_bwd] @ Wd + bd) run
    on-device; host only concatenates the 8 per-core [32, 8] outputs.
"""

import numpy as np

import concourse.bacc as bacc
import concourse.tile as tile
from concourse import mybir
from concourse.bass_utils import run_bass_kernel_spmd
from concourse.masks import make_identity


F32 = mybir.dt.float32
F32R = mybir.dt.float32r
I16 = mybir.dt.int16
AF = mybir.ActivationFunctionType

B, S, E, H, NCLS, VOCAB = 256, 512, 128, 256, 8, 32000
G = 4 * H                      # 1024 gate columns
NCORES = 8
BSH = B // NCORES              # 32 batch rows per chain per core
SPT = 4                        # steps per xT tile (128 gathered rows)
SPB = 16                       # steps per dma_gather block (512 rows)
ROWS_PER_BLK = SPB * BSH       # 512

# column permutation: reference gate order (i,f,g,o) -> kernel order (i,f,o,g)
_PERM = np.concatenate(
    [np.arange(0, 512), np.arange(768, 1024), np.arange(512, 768)]
)


def _emit(tc, ctx, aps, s_steps, has_bias, has_bd):
    nc = tc.nc
    nblk = s_steps // SPB
    ntile = s_steps // SPT

    emb = aps["emb"]
    wcat = aps["wcat"]
    wd = aps["wd"]
    idx = aps["idx"]
    yout = aps["y"]

    consts = ctx.enter_context(tc.tile_pool(name="consts", bufs=1))
    gatp = ctx.enter_context(tc.tile_pool(name="gat", bufs=3))
    xtp = ctx.enter_context(tc.tile_pool(name="xt", bufs=3))
    work = ctx.enter_context(tc.tile_pool(name="work", bufs=3))
    state = ctx.enter_context(tc.tile_pool(name="state", bufs=2))
    pers = ctx.enter_context(tc.tile_pool(name="pers", bufs=1))
    zps = ctx.enter_context(tc.tile_pool(name="zps", bufs=1, space="PSUM"))
    tps = ctx.enter_context(tc.tile_pool(name="tps", bufs=2, space="PSUM"))
    hps = ctx.enter_context(tc.tile_pool(name="hps", bufs=1, space="PSUM"))

    # ---- constants in SBUF ----
    wsb = consts.tile([128, 2, 3, G], F32R)          # [p, dir, kchunk, gates]
    nc.sync.dma_start(out=wsb[:], in_=wcat[:])
    wdsb = consts.tile([128, 4, NCLS], F32R)
    nc.sync.dma_start(out=wdsb[:], in_=wd[:])
    idxsb = consts.tile([128, 2, nblk, ROWS_PER_BLK // 16], I16)
    nc.sync.dma_start(out=idxsb[:], in_=idx[:])
    ident = consts.tile([128, 128], F32)
    make_identity(nc, ident[:])

    if has_bias:
        bsb = consts.tile([1, 2, G], F32R)
        nc.sync.dma_start(out=bsb[:], in_=aps["brow"][:])
    if has_bd:
        bdsb = consts.tile([1, NCLS], F32R)
        nc.sync.dma_start(out=bdsb[:], in_=aps["bdrow"][:])
    if has_bias or has_bd:
        ones1 = consts.tile([1, BSH], F32R)
        nc.vector.memset(ones1[:].bitcast(F32), 1.0)

    # ---- per-chain state ----
    class Chain:
        pass

    chains = []
    for c in range(2):
        st = Chain()
        st.c = c
        st.tgc = pers.tile([BSH, 512], F32, tag=f"tgc{c}")  # [tanh_g | c]
        nc.vector.memset(st.tgc[:], 0.0)
        st.hT = state.tile([128, 64], F32R, tag=f"hT{c}")    # [h-dim chunk, batch]
        nc.vector.memset(st.hT[:].bitcast(F32), 0.0)
        st.gtiles = {}
        st.xtiles = {}
        chains.append(st)

    def emit_gather(st, kb):
        g = gatp.tile([128, ROWS_PER_BLK // 128, E], F32, tag=f"g{st.c}")
        nc.gpsimd.dma_gather(
            out_ap=g[:],
            in_ap=emb[:],
            idxs_ap=idxsb[:, st.c, kb, :],
            num_idxs=ROWS_PER_BLK,
            num_idxs_reg=ROWS_PER_BLK,
            elem_size=E,
            queue_num=st.c,
        )
        st.gtiles[kb] = g

    def emit_xtile(st, n):
        kb, j = divmod(n, SPB // SPT)
        tp = tps.tile([128, 128], F32, tag="tp")
        nc.tensor.transpose(tp[:], st.gtiles[kb][:, j, :], ident[:])
        xT = xtp.tile([128, 128], F32R, tag=f"x{st.c}")
        nc.vector.tensor_copy(xT[:], tp[:])
        st.xtiles[n] = xT
        if j == SPB // SPT - 1:
            del st.gtiles[kb]

    # per-chain z PSUM tiles; x-MMs for step t+1 are emitted during step t
    # so the scheduler can fill PE gaps with ready work.
    ztiles = {}

    def get_ztile(st, t):
        key = (st.c, t)
        if key not in ztiles:
            ztiles[key] = zps.tile(
                [BSH, G], F32, tag=f"z{st.c}", name=f"z{st.c}_{t}"
            )
        return ztiles[key]

    def emit_mms(st, t):
        c = st.c
        if t % SPB == 0:
            kb = t // SPB + 2
            if kb < nblk:
                emit_gather(st, kb)
        if t % SPT == 0:
            n = t // SPT + 1
            if n < ntile:
                emit_xtile(st, n)

        zt = get_ztile(st, t)
        xT = st.xtiles[t // SPT]
        xsl = xT[:, (t % SPT) * BSH : (t % SPT + 1) * BSH]   # [128, 32]
        # bank-major: finish bank 0 first so the i,f sigmoid can start while
        # bank 1's matmuls still stream on PE
        for n in range(2):
            zn = zt[:, 512 * n : 512 * (n + 1)]
            nc.tensor.matmul(
                zn, xsl, wsb[:, c, 0, 512 * n : 512 * (n + 1)],
                start=True, stop=False, skip_group_check=True,
            )
            for k in range(2):
                nc.tensor.matmul(
                    zn, st.hT[:, 32 * k : 32 * (k + 1)],
                    wsb[:, c, 1 + k, 512 * n : 512 * (n + 1)],
                    start=False,
                    stop=(k == 1) and not has_bias,
                    skip_group_check=True,
                )
            if has_bias:
                nc.tensor.matmul(
                    zn, ones1[:], bsb[:, c, 512 * n : 512 * (n + 1)],
                    start=False, stop=True, skip_group_check=True,
                )
        if t % SPT == SPT - 1:
            del st.xtiles[t // SPT]

    def emit_elem(st, t):
        c = st.c
        zt = get_ztile(st, t)
        # gates: per-bank sigmoid (i,f then o) + tanh(g) -> tgc[:, 0:256]
        s = work.tile([BSH, 768], F32, tag=f"s{c}")
        nc.scalar.activation(s[:, 0:512], zt[:, 0:512], AF.Sigmoid)
        nc.scalar.activation(s[:, 512:768], zt[:, 512:768], AF.Sigmoid)
        nc.scalar.activation(st.tgc[:, 0:256], zt[:, 768:1024], AF.Tanh)
        del ztiles[(c, t)]

        # cell: prod = (s_i|s_f) * (tanh_g|c); c' = prod_lo + prod_hi
        prod = work.tile([BSH, 512], F32, tag=f"p{c}")
        nc.vector.tensor_mul(prod[:], s[:, 0:512], st.tgc[:])
        nc.vector.tensor_add(st.tgc[:, 256:512], prod[:, 0:256], prod[:, 256:512])

        tch = work.tile([BSH, H], F32, tag=f"tc{c}")
        nc.scalar.activation(tch[:], st.tgc[:, 256:512], AF.Tanh)
        h = work.tile([BSH, H], F32, tag=f"h{c}")
        nc.vector.tensor_mul(h[:], s[:, 512:768], tch[:])

        # transpose h -> hT [128, 64] for next step's stationary
        hp = hps.tile([128, 64], F32, tag=f"hp{c}")
        nc.tensor.transpose(hp[:, 0:32], h[:, 0:128], ident[0:32, 0:32])
        nc.tensor.transpose(hp[:, 32:64], h[:, 128:256], ident[0:32, 0:32])
        hT = state.tile([128, 64], F32R, tag=f"hT{c}")
        nc.vector.tensor_copy(hT[:], hp[:])
        st.hT = hT

    # prologue: first gathers + first xT tile + step-0 x-MMs per chain
    for st in chains:
        emit_gather(st, 0)
        if nblk > 1:
            emit_gather(st, 1)
        emit_xtile(st, 0)

    # half-step interleave: while chain A's matmuls run, chain B does its
    # previous step's elementwise, and vice versa (anti-phase by construction)
    A, Bc = chains
    for t in range(s_steps):
        emit_mms(A, t)
        if t > 0:
            emit_elem(Bc, t - 1)
        emit_mms(Bc, t)
        emit_elem(A, t)
    emit_elem(Bc, s_steps - 1)

    # ---- final dense: y = [c_fwd | c_bwd] @ Wd (+ bd) ----
    fp = tps.tile([128, 128], F32, tag="tp")
    for st in chains:
        for hh in range(2):
            u = 2 * st.c + hh
            nc.tensor.transpose(
                fp[:, 32 * u : 32 * (u + 1)],
                st.tgc[:, 256 + 128 * hh : 256 + 128 * (hh + 1)],
                ident[0:32, 0:32],
            )
    fT = work.tile([128, 128], F32R, tag="fT")
    nc.vector.tensor_copy(fT[:], fp[:])
    yp = hps.tile([BSH, NCLS], F32, tag="hp0")
    for u in range(4):
        nc.tensor.matmul(
            yp[:], fT[:, 32 * u : 32 * (u + 1)], wdsb[:, u, :],
            start=(u == 0), stop=(u == 3 and not has_bd),
        )
    if has_bd:
        nc.tensor.matmul(yp[:], ones1[:], bdsb[:], start=False, stop=True)
    ysb = work.tile([BSH, NCLS], F32, tag="y")
    nc.vector.tensor_copy(ysb[:], yp[:])
    nc.sync.dma_start(out=yout[:], in_=ysb[:])


def build(s_steps=S, has_bias=False, has_bd=False):
    """Build + compile the SPMD program. Returns the Bacc instance."""
    nblk = s_steps // SPB
    nc = bacc.Bacc("TRN2", debug=False, num_devices=NCORES, num_swdge_queues=2)
    aps = {
        "emb": nc.dram_tensor("emb", [VOCAB, E], F32, kind="ExternalInput").ap(),
        "wcat": nc.dram_tensor("wcat", [128, 2, 3, G], F32R, kind="ExternalInput").ap(),
        "wd": nc.dram_tensor("wd", [128, 4, NCLS], F32R, kind="ExternalInput").ap(),
        "idx": nc.dram_tensor(
            "idx", [128, 2, nblk, ROWS_PER_BLK // 16], I16, kind="ExternalInput"
        ).ap(),
        "y": nc.dram_tensor("y", [BSH, NCLS], F32, kind="ExternalOutput").ap(),
    }
    if has_bias:
        aps["brow"] = nc.dram_tensor("brow", [1, 2, G], F32R, kind="ExternalInput").ap()
    if has_bd:
        aps["bdrow"] = nc.dram_tensor("bdrow", [1, NCLS], F32R, kind="ExternalInput").ap()
    from contextlib import ExitStack
    with tile.TileContext(nc) as tc, ExitStack() as ctx:
        _emit(tc, ctx, aps, s_steps, has_bias, has_bd)
    nc.compile()
    return nc


def prep_inputs(tokens, emb, Wi_f, Wh_f, b_f, Wi_b, Wh_b, b_b, Wd, bd,
                s_steps=S, has_bias=False, has_bd=False):
    """Host-side shard/layout prep. Returns in_maps for run_bass_kernel_spmd."""
    emb = np.ascontiguousarray(np.asarray(emb, dtype=np.float32))
    tokens = np.asarray(tokens)

    def wprep(Wi, Wh):
        Wi_p = np.asarray(Wi, np.float32)[:, _PERM]
        Wh_p = np.asarray(Wh, np.float32)[:, _PERM]
        return np.stack([Wi_p, Wh_p[:128], Wh_p[128:]], axis=1)  # [128, 3, G]

    wcat = np.ascontiguousarray(
        np.stack([wprep(Wi_f, Wh_f), wprep(Wi_b, Wh_b)], axis=1)
    )  # [128, 2, 3, G]

    Wd = np.asarray(Wd, np.float32)  # [2H, NCLS]
    wdcat = np.ascontiguousarray(
        np.stack([Wd[128 * u : 128 * (u + 1)] for u in range(4)], axis=1)
    )  # [128, 4, NCLS]

    nblk = s_steps // SPB
    in_maps = []
    for k in range(NCORES):
        tf = tokens[BSH * k : BSH * (k + 1), :s_steps]
        tb = tf[:, ::-1]
        idx_host = np.zeros((128, 2, nblk, ROWS_PER_BLK // 16), np.int16)
        for c, tk in ((0, tf), (1, tb)):
            for kb in range(nblk):
                vals = np.ascontiguousarray(
                    tk[:, SPB * kb : SPB * (kb + 1)].T
                ).reshape(-1)  # i = BSH*t' + b
                # wrapped [16, n/16] pattern, replicated across all 8
                # gpsimd-core stripes (HW reads its own stripe; sim reads 0:16)
                idx_host[:, c, kb, :] = np.tile(
                    vals.reshape(-1, 16).T.astype(np.int16), (8, 1)
                )
        m = {
            "emb": emb,
            "wcat": wcat,
            "wd": wdcat,
            "idx": idx_host,
        }
        if has_bias:
            m["brow"] = np.stack(
                [np.asarray(b_f, np.float32)[_PERM], np.asarray(b_b, np.float32)[_PERM]]
            ).reshape(1, 2, G)
        if has_bd:
            m["bdrow"] = np.asarray(bd, np.float32).reshape(1, NCLS)
        in_maps.append(m)
    return in_maps


_CACHE = {}


def kernel(tokens, emb, Wi_f, Wh_f, b_f, Wi_b, Wh_b, b_b, Wd, bd, train=0):
    tokens = np.asarray(tokens)
    assert tokens.shape == (B, S) and int(tokens.max()) < 32768
    has_bias = bool(np.any(np.asarray(b_f)) or np.any(np.asarray(b_b)))
    has_bd = bool(np.any(np.asarray(bd)))
    key = (has_bias, has_bd)
    if key not in _CACHE:
        _CACHE[key] = build(S, has_bias, has_bd)
    nc = _CACHE[key]
    in_maps = prep_inputs(
        tokens, emb, Wi_f, Wh_f, b_f, Wi_b, Wh_b, b_b, Wd, bd,
        s_steps=S, has_bias=has_bias, has_bd=has_bd,
    )
    res = run_bass_kernel_spmd(nc, in_maps, core_ids=list(range(NCORES)))
    y = np.concatenate([res.results[k]["y"] for k in range(NCORES)], axis=0)
    return y.astype(np.float32)


# revision 16
# speedup vs baseline: 1.1191x; 1.1191x over previous
"""Trainium2 Bass kernel: BiLSTM classifier (nn_BiLSTMClassifier_11063835755286).

Strategy (8 NeuronCores, pure data-parallel SPMD, no collectives):
  - core k owns batch rows [32k, 32k+32) and runs TWO independent LSTM chains
    (forward tokens + time-flipped tokens), pipelined against each other so
    PE / ACT / DVE overlap across the 512 sequential steps.
  - layout: batch on partitions, gates on the free dim ("form A").
    z_t = [x_t, h_{t-1}] @ [Wi; Wh]  via a 3-chunk augmented-K accumulation
    group in PSUM (no separate input-projection pass, no xp add).
  - embedding rows are fetched with dma_gather (rows -> partitions) and
    transposed to E-on-partitions with PE transposes, 4 steps per tile,
    prefetched ahead of the recurrence.
  - gate order is host-permuted to (i, f, o, g) so one sigmoid covers
    [0:768) and the cell update runs as two fused tensor_tensor ops.
  - final feature transpose + tiny dense (y = [c_fwd|c_bwd] @ Wd + bd) run
    on-device; host only concatenates the 8 per-core [32, 8] outputs.
"""

import numpy as np

import concourse.bacc as bacc
import concourse.tile as tile
from concourse import mybir
from concourse.bass_utils import run_bass_kernel_spmd
from concourse.masks import make_identity


F32 = mybir.dt.float32
F32R = mybir.dt.float32r
I16 = mybir.dt.int16
AF = mybir.ActivationFunctionType

B, S, E, H, NCLS, VOCAB = 256, 512, 128, 256, 8, 32000
G = 4 * H                      # 1024 gate columns
NCORES = 8
BSH = B // NCORES              # 32 batch rows per chain per core
SPT = 4                        # steps per xT tile (128 gathered rows)
SPB = 16                       # steps per dma_gather block (512 rows)
ROWS_PER_BLK = SPB * BSH       # 512

# column permutation: reference gate order (i,f,g,o) -> kernel order (i,f,o,g)
_PERM = np.concatenate(
    [np.arange(0, 512), np.arange(768, 1024), np.arange(512, 768)]
)


def _emit(tc, ctx, aps, s_steps, has_bias, has_bd):
    nc = tc.nc
    nblk = s_steps // SPB
    ntile = s_steps // SPT

    emb = aps["emb"]
    wcat = aps["wcat"]
    wd = aps["wd"]
    idx = aps["idx"]
    yout = aps["y"]

    consts = ctx.enter_context(tc.tile_pool(name="consts", bufs=1))
    gatp = ctx.enter_context(tc.tile_pool(name="gat", bufs=3))
    xtp = ctx.enter_context(tc.tile_pool(name="xt", bufs=3))
    work = ctx.enter_context(tc.tile_pool(name="work", bufs=3))
    state = ctx.enter_context(tc.tile_pool(name="state", bufs=2))
    pers = ctx.enter_context(tc.tile_pool(name="pers", bufs=1))
    zps = ctx.enter_context(tc.tile_pool(name="zps", bufs=1, space="PSUM"))
    tps = ctx.enter_context(tc.tile_pool(name="tps", bufs=2, space="PSUM"))
    hps = ctx.enter_context(tc.tile_pool(name="hps", bufs=1, space="PSUM"))

    # ---- constants in SBUF ----
    wsb = consts.tile([128, 2, 3, G], F32R)          # [p, dir, kchunk, gates]
    nc.sync.dma_start(out=wsb[:], in_=wcat[:])
    wdsb = consts.tile([128, 4, NCLS], F32R)
    nc.sync.dma_start(out=wdsb[:], in_=wd[:])
    idxsb = consts.tile([128, 2, nblk, ROWS_PER_BLK // 16], I16)
    nc.sync.dma_start(out=idxsb[:], in_=idx[:])
    ident = consts.tile([128, 128], F32)
    make_identity(nc, ident[:])

    if has_bias:
        bsb = consts.tile([1, 2, G], F32R)
        nc.sync.dma_start(out=bsb[:], in_=aps["brow"][:])
    if has_bd:
        bdsb = consts.tile([1, NCLS], F32R)
        nc.sync.dma_start(out=bdsb[:], in_=aps["bdrow"][:])
    if has_bias or has_bd:
        ones1 = consts.tile([1, BSH], F32R)
        nc.vector.memset(ones1[:].bitcast(F32), 1.0)

    # ---- per-chain state ----
    class Chain:
        pass

    chains = []
    for c in range(2):
        st = Chain()
        st.c = c
        st.tgc = pers.tile([BSH, 512], F32, tag=f"tgc{c}")  # [tanh_g | c]
        nc.vector.memset(st.tgc[:], 0.0)
        st.hT = state.tile([128, 64], F32R, tag=f"hT{c}")    # [h-dim chunk, batch]
        nc.vector.memset(st.hT[:].bitcast(F32), 0.0)
        st.gtiles = {}
        st.xtiles = {}
        chains.append(st)

    def emit_gather(st, kb):
        g = gatp.tile([128, ROWS_PER_BLK // 128, E], F32, tag=f"g{st.c}")
        nc.gpsimd.dma_gather(
            out_ap=g[:],
            in_ap=emb[:],
            idxs_ap=idxsb[:, st.c, kb, :],
            num_idxs=ROWS_PER_BLK,
            num_idxs_reg=ROWS_PER_BLK,
            elem_size=E,
            queue_num=st.c,
        )
        st.gtiles[kb] = g

    def emit_xtile(st, n):
        kb, j = divmod(n, SPB // SPT)
        tp = tps.tile([128, 128], F32, tag="tp")
        nc.tensor.transpose(tp[:], st.gtiles[kb][:, j, :], ident[:])
        xT = xtp.tile([128, 128], F32R, tag=f"x{st.c}")
        nc.vector.tensor_copy(xT[:], tp[:])
        st.xtiles[n] = xT
        if j == SPB // SPT - 1:
            del st.gtiles[kb]

    # per-chain z PSUM tiles; x-MMs for step t+1 are emitted during step t
    # so the scheduler can fill PE gaps with ready work.
    ztiles = {}

    def get_ztile(st, t):
        key = (st.c, t)
        if key not in ztiles:
            ztiles[key] = zps.tile(
                [BSH, G], F32, tag=f"z{st.c}", name=f"z{st.c}_{t}"
            )
        return ztiles[key]

    def emit_mms(st, t):
        c = st.c
        if t % SPB == 0:
            kb = t // SPB + 2
            if kb < nblk:
                emit_gather(st, kb)
        if t % SPT == 0:
            n = t // SPT + 1
            if n < ntile:
                emit_xtile(st, n)

        zt = get_ztile(st, t)
        xT = st.xtiles[t // SPT]
        xsl = xT[:, (t % SPT) * BSH : (t % SPT + 1) * BSH]   # [128, 32]
        # x-projection first (no recurrence dependency), then h-matmuls with
        # bank 0 completing first so the i,f sigmoid starts 2 MMs early
        for n in range(2):
            nc.tensor.matmul(
                zt[:, 512 * n : 512 * (n + 1)],
                xsl, wsb[:, c, 0, 512 * n : 512 * (n + 1)],
                start=True, stop=False, skip_group_check=True,
            )
        for n in range(2):
            for k in range(2):
                nc.tensor.matmul(
                    zt[:, 512 * n : 512 * (n + 1)],
                    st.hT[:, 32 * k : 32 * (k + 1)],
                    wsb[:, c, 1 + k, 512 * n : 512 * (n + 1)],
                    start=False,
                    stop=(k == 1) and not has_bias,
                    skip_group_check=True,
                )
            if has_bias:
                nc.tensor.matmul(
                    zt[:, 512 * n : 512 * (n + 1)],
                    ones1[:], bsb[:, c, 512 * n : 512 * (n + 1)],
                    start=False, stop=True, skip_group_check=True,
                )
        if t % SPT == SPT - 1:
            del st.xtiles[t // SPT]

    def emit_elem(st, t):
        c = st.c
        zt = get_ztile(st, t)
        # gates: per-bank sigmoid (i,f then o) + tanh(g) -> tgc[:, 0:256]
        s = work.tile([BSH, 768], F32, tag=f"s{c}")
        nc.scalar.activation(s[:, 0:512], zt[:, 0:512], AF.Sigmoid)
        nc.scalar.activation(s[:, 512:768], zt[:, 512:768], AF.Sigmoid)
        nc.scalar.activation(st.tgc[:, 0:256], zt[:, 768:1024], AF.Tanh)
        del ztiles[(c, t)]

        # cell: prod = (s_i|s_f) * (tanh_g|c); c' = prod_lo + prod_hi
        prod = work.tile([BSH, 512], F32, tag=f"p{c}")
        nc.vector.tensor_mul(prod[:], s[:, 0:512], st.tgc[:])
        nc.vector.tensor_add(st.tgc[:, 256:512], prod[:, 0:256], prod[:, 256:512])

        tch = work.tile([BSH, H], F32, tag=f"tc{c}")
        nc.scalar.activation(tch[:], st.tgc[:, 256:512], AF.Tanh)
        h = work.tile([BSH, H], F32, tag=f"h{c}")
        nc.vector.tensor_mul(h[:], s[:, 512:768], tch[:])

        # transpose h -> hT [128, 64] for next step's stationary
        hp = hps.tile([128, 64], F32, tag=f"hp{c}")
        nc.tensor.transpose(hp[:, 0:32], h[:, 0:128], ident[0:32, 0:32])
        nc.tensor.transpose(hp[:, 32:64], h[:, 128:256], ident[0:32, 0:32])
        hT = state.tile([128, 64], F32R, tag=f"hT{c}")
        nc.vector.tensor_copy(hT[:], hp[:])
        st.hT = hT

    # prologue: first gathers + first xT tile + step-0 x-MMs per chain
    for st in chains:
        emit_gather(st, 0)
        if nblk > 1:
            emit_gather(st, 1)
        emit_xtile(st, 0)

    # half-step interleave: while chain A's matmuls run, chain B does its
    # previous step's elementwise, and vice versa (anti-phase by construction)
    A, Bc = chains
    for t in range(s_steps):
        emit_mms(A, t)
        if t > 0:
            emit_elem(Bc, t - 1)
        emit_mms(Bc, t)
        emit_elem(A, t)
    emit_elem(Bc, s_steps - 1)

    # ---- final dense: y = [c_fwd | c_bwd] @ Wd (+ bd) ----
    fp = tps.tile([128, 128], F32, tag="tp")
    for st in chains:
        for hh in range(2):
            u = 2 * st.c + hh
            nc.tensor.transpose(
                fp[:, 32 * u : 32 * (u + 1)],
                st.tgc[:, 256 + 128 * hh : 256 + 128 * (hh + 1)],
                ident[0:32, 0:32],
            )
    fT = work.tile([128, 128], F32R, tag="fT")
    nc.vector.tensor_copy(fT[:], fp[:])
    yp = hps.tile([BSH, NCLS], F32, tag="hp0")
    for u in range(4):
        nc.tensor.matmul(
            yp[:], fT[:, 32 * u : 32 * (u + 1)], wdsb[:, u, :],
            start=(u == 0), stop=(u == 3 and not has_bd),
        )
    if has_bd:
        nc.tensor.matmul(yp[:], ones1[:], bdsb[:], start=False, stop=True)
    ysb = work.tile([BSH, NCLS], F32, tag="y")
    nc.vector.tensor_copy(ysb[:], yp[:])
    nc.sync.dma_start(out=yout[:], in_=ysb[:])


def build(s_steps=S, has_bias=False, has_bd=False):
    """Build + compile the SPMD program. Returns the Bacc instance."""
    nblk = s_steps // SPB
    nc = bacc.Bacc("TRN2", debug=False, num_devices=NCORES, num_swdge_queues=2)
    aps = {
        "emb": nc.dram_tensor("emb", [VOCAB, E], F32, kind="ExternalInput").ap(),
        "wcat": nc.dram_tensor("wcat", [128, 2, 3, G], F32R, kind="ExternalInput").ap(),
        "wd": nc.dram_tensor("wd", [128, 4, NCLS], F32R, kind="ExternalInput").ap(),
        "idx": nc.dram_tensor(
            "idx", [128, 2, nblk, ROWS_PER_BLK // 16], I16, kind="ExternalInput"
        ).ap(),
        "y": nc.dram_tensor("y", [BSH, NCLS], F32, kind="ExternalOutput").ap(),
    }
    if has_bias:
        aps["brow"] = nc.dram_tensor("brow", [1, 2, G], F32R, kind="ExternalInput").ap()
    if has_bd:
        aps["bdrow"] = nc.dram_tensor("bdrow", [1, NCLS], F32R, kind="ExternalInput").ap()
    from contextlib import ExitStack
    with tile.TileContext(nc) as tc, ExitStack() as ctx:
        _emit(tc, ctx, aps, s_steps, has_bias, has_bd)
    nc.compile()
    return nc


def prep_inputs(tokens, emb, Wi_f, Wh_f, b_f, Wi_b, Wh_b, b_b, Wd, bd,
                s_steps=S, has_bias=False, has_bd=False):
    """Host-side shard/layout prep. Returns in_maps for run_bass_kernel_spmd."""
    emb = np.ascontiguousarray(np.asarray(emb, dtype=np.float32))
    tokens = np.asarray(tokens)

    def wprep(Wi, Wh):
        Wi_p = np.asarray(Wi, np.float32)[:, _PERM]
        Wh_p = np.asarray(Wh, np.float32)[:, _PERM]
        return np.stack([Wi_p, Wh_p[:128], Wh_p[128:]], axis=1)  # [128, 3, G]

    wcat = np.ascontiguousarray(
        np.stack([wprep(Wi_f, Wh_f), wprep(Wi_b, Wh_b)], axis=1)
    )  # [128, 2, 3, G]

    Wd = np.asarray(Wd, np.float32)  # [2H, NCLS]
    wdcat = np.ascontiguousarray(
        np.stack([Wd[128 * u : 128 * (u + 1)] for u in range(4)], axis=1)
    )  # [128, 4, NCLS]

    nblk = s_steps // SPB
    in_maps = []
    for k in range(NCORES):
        tf = tokens[BSH * k : BSH * (k + 1), :s_steps]
        tb = tf[:, ::-1]
        idx_host = np.zeros((128, 2, nblk, ROWS_PER_BLK // 16), np.int16)
        for c, tk in ((0, tf), (1, tb)):
            for kb in range(nblk):
                vals = np.ascontiguousarray(
                    tk[:, SPB * kb : SPB * (kb + 1)].T
                ).reshape(-1)  # i = BSH*t' + b
                # wrapped [16, n/16] pattern, replicated across all 8
                # gpsimd-core stripes (HW reads its own stripe; sim reads 0:16)
                idx_host[:, c, kb, :] = np.tile(
                    vals.reshape(-1, 16).T.astype(np.int16), (8, 1)
                )
        m = {
            "emb": emb,
            "wcat": wcat,
            "wd": wdcat,
            "idx": idx_host,
        }
        if has_bias:
            m["brow"] = np.stack(
                [np.asarray(b_f, np.float32)[_PERM], np.asarray(b_b, np.float32)[_PERM]]
            ).reshape(1, 2, G)
        if has_bd:
            m["bdrow"] = np.asarray(bd, np.float32).reshape(1, NCLS)
        in_maps.append(m)
    return in_maps


_CACHE = {}


def kernel(tokens, emb, Wi_f, Wh_f, b_f, Wi_b, Wh_b, b_b, Wd, bd, train=0):
    tokens = np.asarray(tokens)
    assert tokens.shape == (B, S) and int(tokens.max()) < 32768
    has_bias = bool(np.any(np.asarray(b_f)) or np.any(np.asarray(b_b)))
    has_bd = bool(np.any(np.asarray(bd)))
    key = (has_bias, has_bd)
    if key not in _CACHE:
        _CACHE[key] = build(S, has_bias, has_bd)
    nc = _CACHE[key]
    in_maps = prep_inputs(
        tokens, emb, Wi_f, Wh_f, b_f, Wi_b, Wh_b, b_b, Wd, bd,
        s_steps=S, has_bias=has_bias, has_bd=has_bd,
    )
    res = run_bass_kernel_spmd(nc, in_maps, core_ids=list(range(NCORES)))
    y = np.concatenate([res.results[k]["y"] for k in range(NCORES)], axis=0)
    return y.astype(np.float32)


# revision 17
# speedup vs baseline: 1.3710x; 1.2251x over previous
"""Trainium2 Bass kernel: BiLSTM classifier (nn_BiLSTMClassifier_11063835755286).

Strategy (8 NeuronCores, pure data-parallel SPMD, no collectives):
  - core k owns batch rows [32k, 32k+32) and runs TWO independent LSTM chains
    (forward tokens + time-flipped tokens), pipelined against each other so
    PE / ACT / DVE overlap across the 512 sequential steps.
  - layout: batch on partitions, gates on the free dim ("form A").
    z_t = [x_t, h_{t-1}] @ [Wi; Wh]  via a 3-chunk augmented-K accumulation
    group in PSUM (no separate input-projection pass, no xp add).
  - embedding rows are fetched with dma_gather (rows -> partitions) and
    transposed to E-on-partitions with PE transposes, 4 steps per tile,
    prefetched ahead of the recurrence.
  - gate order is host-permuted to (i, f, o, g) so one sigmoid covers
    [0:768) and the cell update runs as two fused tensor_tensor ops.
  - final feature transpose + tiny dense (y = [c_fwd|c_bwd] @ Wd + bd) run
    on-device; host only concatenates the 8 per-core [32, 8] outputs.
"""

import numpy as np

import concourse.bacc as bacc
import concourse.tile as tile
from concourse import mybir
from concourse.bass_utils import run_bass_kernel_spmd
from concourse.masks import make_identity


F32 = mybir.dt.float32
F32R = mybir.dt.float32r
I16 = mybir.dt.int16
AF = mybir.ActivationFunctionType

B, S, E, H, NCLS, VOCAB = 256, 512, 128, 256, 8, 32000
G = 4 * H                      # 1024 gate columns
NCORES = 8
BSH = B // NCORES              # 32 batch rows per chain per core
SPT = 4                        # steps per xT tile (128 gathered rows)
SPB = 16                       # steps per dma_gather block (512 rows)
ROWS_PER_BLK = SPB * BSH       # 512

# column permutation: reference gate order (i,f,g,o) -> kernel order (g,f,i,o).
# bank 0 (cols 0:512) = g,f so tanh(g)/sigmoid(f) start while bank 1 streams;
# bank 1 (cols 512:1024) = i,o in one sigmoid call.
_PERM = np.concatenate(
    [np.arange(512, 768), np.arange(256, 512),
     np.arange(0, 256), np.arange(768, 1024)]
)


def _emit(tc, ctx, aps, s_steps, has_bias, has_bd):
    nc = tc.nc
    nblk = s_steps // SPB
    ntile = s_steps // SPT

    emb = aps["emb"]
    wcat = aps["wcat"]
    wd = aps["wd"]
    idx = aps["idx"]
    yout = aps["y"]

    consts = ctx.enter_context(tc.tile_pool(name="consts", bufs=1))
    gatp = ctx.enter_context(tc.tile_pool(name="gat", bufs=3))
    xtp = ctx.enter_context(tc.tile_pool(name="xt", bufs=3))
    work = ctx.enter_context(tc.tile_pool(name="work", bufs=3))
    state = ctx.enter_context(tc.tile_pool(name="state", bufs=2))
    pers = ctx.enter_context(tc.tile_pool(name="pers", bufs=1))
    zps = ctx.enter_context(tc.tile_pool(name="zps", bufs=1, space="PSUM"))
    tps = ctx.enter_context(tc.tile_pool(name="tps", bufs=2, space="PSUM"))
    hps = ctx.enter_context(tc.tile_pool(name="hps", bufs=1, space="PSUM"))

    # ---- constants in SBUF ----
    wsb = consts.tile([128, 2, 3, G], F32R)          # [p, dir, kchunk, gates]
    nc.sync.dma_start(out=wsb[:], in_=wcat[:])
    wdsb = consts.tile([128, 4, NCLS], F32R)
    nc.sync.dma_start(out=wdsb[:], in_=wd[:])
    idxsb = consts.tile([128, 2, nblk, ROWS_PER_BLK // 16], I16)
    nc.sync.dma_start(out=idxsb[:], in_=idx[:])
    ident = consts.tile([128, 128], F32)
    make_identity(nc, ident[:])

    if has_bias:
        bsb = consts.tile([1, 2, G], F32R)
        nc.sync.dma_start(out=bsb[:], in_=aps["brow"][:])
    if has_bd:
        bdsb = consts.tile([1, NCLS], F32R)
        nc.sync.dma_start(out=bdsb[:], in_=aps["bdrow"][:])
    if has_bias or has_bd:
        ones1 = consts.tile([1, BSH], F32R)
        nc.vector.memset(ones1[:].bitcast(F32), 1.0)

    # ---- per-chain state ----
    class Chain:
        pass

    chains = []
    for c in range(2):
        st = Chain()
        st.c = c
        st.tgc = pers.tile([BSH, 512], F32, tag=f"tgc{c}")  # [tanh_g | c]
        nc.vector.memset(st.tgc[:], 0.0)
        st.hT = state.tile([128, 64], F32R, tag=f"hT{c}")    # [h-dim chunk, batch]
        nc.vector.memset(st.hT[:].bitcast(F32), 0.0)
        st.gtiles = {}
        st.xtiles = {}
        chains.append(st)

    def emit_gather(st, kb):
        g = gatp.tile([128, ROWS_PER_BLK // 128, E], F32, tag=f"g{st.c}")
        nc.gpsimd.dma_gather(
            out_ap=g[:],
            in_ap=emb[:],
            idxs_ap=idxsb[:, st.c, kb, :],
            num_idxs=ROWS_PER_BLK,
            num_idxs_reg=ROWS_PER_BLK,
            elem_size=E,
            queue_num=st.c,
        )
        st.gtiles[kb] = g

    def emit_xtile(st, n):
        kb, j = divmod(n, SPB // SPT)
        tp = tps.tile([128, 128], F32, tag="tp")
        nc.tensor.transpose(tp[:], st.gtiles[kb][:, j, :], ident[:])
        xT = xtp.tile([128, 128], F32R, tag=f"x{st.c}")
        nc.vector.tensor_copy(xT[:], tp[:])
        st.xtiles[n] = xT
        if j == SPB // SPT - 1:
            del st.gtiles[kb]

    # per-chain z PSUM tiles; x-MMs for step t+1 are emitted during step t
    # so the scheduler can fill PE gaps with ready work.
    ztiles = {}

    def get_ztile(st, t):
        key = (st.c, t)
        if key not in ztiles:
            ztiles[key] = zps.tile(
                [BSH, G], F32, tag=f"z{st.c}", name=f"z{st.c}_{t}"
            )
        return ztiles[key]

    def emit_mms(st, t):
        c = st.c
        if t % SPB == 0:
            kb = t // SPB + 2
            if kb < nblk:
                emit_gather(st, kb)
        if t % SPT == 0:
            n = t // SPT + 1
            if n < ntile:
                emit_xtile(st, n)

        zt = get_ztile(st, t)
        xT = st.xtiles[t // SPT]
        xsl = xT[:, (t % SPT) * BSH : (t % SPT + 1) * BSH]   # [128, 32]
        # x-projection first (no recurrence dependency), then h-matmuls with
        # bank 0 completing first so the i,f sigmoid starts 2 MMs early
        for n in range(2):
            nc.tensor.matmul(
                zt[:, 512 * n : 512 * (n + 1)],
                xsl, wsb[:, c, 0, 512 * n : 512 * (n + 1)],
                start=True, stop=False, skip_group_check=True,
            )
        for n in range(2):
            for k in range(2):
                nc.tensor.matmul(
                    zt[:, 512 * n : 512 * (n + 1)],
                    st.hT[:, 32 * k : 32 * (k + 1)],
                    wsb[:, c, 1 + k, 512 * n : 512 * (n + 1)],
                    start=False,
                    stop=(k == 1) and not has_bias,
                    skip_group_check=True,
                )
            if has_bias:
                nc.tensor.matmul(
                    zt[:, 512 * n : 512 * (n + 1)],
                    ones1[:], bsb[:, c, 512 * n : 512 * (n + 1)],
                    start=False, stop=True, skip_group_check=True,
                )
        if t % SPT == SPT - 1:
            del st.xtiles[t // SPT]

    def emit_elem(st, t):
        c = st.c
        zt = get_ztile(st, t)
        # bank0 gates (early, overlap bank1 matmuls): tanh(g), sigmoid(f)
        sf = work.tile([BSH, H], F32, tag=f"sf{c}")
        nc.scalar.activation(st.tgc[:, 0:256], zt[:, 0:256], AF.Tanh)
        nc.scalar.activation(sf[:], zt[:, 256:512], AF.Sigmoid)
        # prod_f = sigmoid(f) * c uses last step's c -> runs during MM phase
        pf = work.tile([BSH, H], F32, tag=f"pf{c}")
        nc.vector.tensor_mul(pf[:], sf[:], st.tgc[:, 256:512])
        # bank1 gates: one sigmoid over (i, o)
        sio = work.tile([BSH, 512], F32, tag=f"sio{c}")
        nc.scalar.activation(sio[:], zt[:, 512:1024], AF.Sigmoid)
        del ztiles[(c, t)]

        pi = work.tile([BSH, H], F32, tag=f"pi{c}")
        nc.vector.tensor_mul(pi[:], sio[:, 0:256], st.tgc[:, 0:256])
        nc.vector.tensor_add(st.tgc[:, 256:512], pf[:], pi[:])

        tch = work.tile([BSH, H], F32, tag=f"tc{c}")
        nc.scalar.activation(tch[:], st.tgc[:, 256:512], AF.Tanh)
        h = work.tile([BSH, H], F32, tag=f"h{c}")
        nc.vector.tensor_mul(h[:], sio[:, 256:512], tch[:])

        # transpose h -> hT [128, 64] for next step's stationary
        hp = hps.tile([128, 64], F32, tag=f"hp{c}")
        nc.tensor.transpose(hp[:, 0:32], h[:, 0:128], ident[0:32, 0:32])
        nc.tensor.transpose(hp[:, 32:64], h[:, 128:256], ident[0:32, 0:32])
        hT = state.tile([128, 64], F32R, tag=f"hT{c}")
        nc.vector.tensor_copy(hT[:], hp[:])
        st.hT = hT

    # prologue: first gathers + first xT tile + step-0 x-MMs per chain
    for st in chains:
        emit_gather(st, 0)
        if nblk > 1:
            emit_gather(st, 1)
        emit_xtile(st, 0)

    # half-step interleave: while chain A's matmuls run, chain B does its
    # previous step's elementwise, and vice versa (anti-phase by construction)
    A, Bc = chains
    for t in range(s_steps):
        emit_mms(A, t)
        if t > 0:
            emit_elem(Bc, t - 1)
        emit_mms(Bc, t)
        emit_elem(A, t)
    emit_elem(Bc, s_steps - 1)

    # ---- final dense: y = [c_fwd | c_bwd] @ Wd (+ bd) ----
    fp = tps.tile([128, 128], F32, tag="tp")
    for st in chains:
        for hh in range(2):
            u = 2 * st.c + hh
            nc.tensor.transpose(
                fp[:, 32 * u : 32 * (u + 1)],
                st.tgc[:, 256 + 128 * hh : 256 + 128 * (hh + 1)],
                ident[0:32, 0:32],
            )
    fT = work.tile([128, 128], F32R, tag="fT")
    nc.vector.tensor_copy(fT[:], fp[:])
    yp = hps.tile([BSH, NCLS], F32, tag="hp0")
    for u in range(4):
        nc.tensor.matmul(
            yp[:], fT[:, 32 * u : 32 * (u + 1)], wdsb[:, u, :],
            start=(u == 0), stop=(u == 3 and not has_bd),
        )
    if has_bd:
        nc.tensor.matmul(yp[:], ones1[:], bdsb[:], start=False, stop=True)
    ysb = work.tile([BSH, NCLS], F32, tag="y")
    nc.vector.tensor_copy(ysb[:], yp[:])
    nc.sync.dma_start(out=yout[:], in_=ysb[:])


def build(s_steps=S, has_bias=False, has_bd=False):
    """Build + compile the SPMD program. Returns the Bacc instance."""
    nblk = s_steps // SPB
    nc = bacc.Bacc("TRN2", debug=False, num_devices=NCORES, num_swdge_queues=2)
    aps = {
        "emb": nc.dram_tensor("emb", [VOCAB, E], F32, kind="ExternalInput").ap(),
        "wcat": nc.dram_tensor("wcat", [128, 2, 3, G], F32R, kind="ExternalInput").ap(),
        "wd": nc.dram_tensor("wd", [128, 4, NCLS], F32R, kind="ExternalInput").ap(),
        "idx": nc.dram_tensor(
            "idx", [128, 2, nblk, ROWS_PER_BLK // 16], I16, kind="ExternalInput"
        ).ap(),
        "y": nc.dram_tensor("y", [BSH, NCLS], F32, kind="ExternalOutput").ap(),
    }
    if has_bias:
        aps["brow"] = nc.dram_tensor("brow", [1, 2, G], F32R, kind="ExternalInput").ap()
    if has_bd:
        aps["bdrow"] = nc.dram_tensor("bdrow", [1, NCLS], F32R, kind="ExternalInput").ap()
    from contextlib import ExitStack
    with tile.TileContext(nc) as tc, ExitStack() as ctx:
        _emit(tc, ctx, aps, s_steps, has_bias, has_bd)
    nc.compile()
    return nc


def prep_inputs(tokens, emb, Wi_f, Wh_f, b_f, Wi_b, Wh_b, b_b, Wd, bd,
                s_steps=S, has_bias=False, has_bd=False):
    """Host-side shard/layout prep. Returns in_maps for run_bass_kernel_spmd."""
    emb = np.ascontiguousarray(np.asarray(emb, dtype=np.float32))
    tokens = np.asarray(tokens)

    def wprep(Wi, Wh):
        Wi_p = np.asarray(Wi, np.float32)[:, _PERM]
        Wh_p = np.asarray(Wh, np.float32)[:, _PERM]
        return np.stack([Wi_p, Wh_p[:128], Wh_p[128:]], axis=1)  # [128, 3, G]

    wcat = np.ascontiguousarray(
        np.stack([wprep(Wi_f, Wh_f), wprep(Wi_b, Wh_b)], axis=1)
    )  # [128, 2, 3, G]

    Wd = np.asarray(Wd, np.float32)  # [2H, NCLS]
    wdcat = np.ascontiguousarray(
        np.stack([Wd[128 * u : 128 * (u + 1)] for u in range(4)], axis=1)
    )  # [128, 4, NCLS]

    nblk = s_steps // SPB
    in_maps = []
    for k in range(NCORES):
        tf = tokens[BSH * k : BSH * (k + 1), :s_steps]
        tb = tf[:, ::-1]
        idx_host = np.zeros((128, 2, nblk, ROWS_PER_BLK // 16), np.int16)
        for c, tk in ((0, tf), (1, tb)):
            for kb in range(nblk):
                vals = np.ascontiguousarray(
                    tk[:, SPB * kb : SPB * (kb + 1)].T
                ).reshape(-1)  # i = BSH*t' + b
                # wrapped [16, n/16] pattern, replicated across all 8
                # gpsimd-core stripes (HW reads its own stripe; sim reads 0:16)
                idx_host[:, c, kb, :] = np.tile(
                    vals.reshape(-1, 16).T.astype(np.int16), (8, 1)
                )
        m = {
            "emb": emb,
            "wcat": wcat,
            "wd": wdcat,
            "idx": idx_host,
        }
        if has_bias:
            m["brow"] = np.stack(
                [np.asarray(b_f, np.float32)[_PERM], np.asarray(b_b, np.float32)[_PERM]]
            ).reshape(1, 2, G)
        if has_bd:
            m["bdrow"] = np.asarray(bd, np.float32).reshape(1, NCLS)
        in_maps.append(m)
    return in_maps


_CACHE = {}


def kernel(tokens, emb, Wi_f, Wh_f, b_f, Wi_b, Wh_b, b_b, Wd, bd, train=0):
    tokens = np.asarray(tokens)
    assert tokens.shape == (B, S) and int(tokens.max()) < 32768
    has_bias = bool(np.any(np.asarray(b_f)) or np.any(np.asarray(b_b)))
    has_bd = bool(np.any(np.asarray(bd)))
    key = (has_bias, has_bd)
    if key not in _CACHE:
        _CACHE[key] = build(S, has_bias, has_bd)
    nc = _CACHE[key]
    in_maps = prep_inputs(
        tokens, emb, Wi_f, Wh_f, b_f, Wi_b, Wh_b, b_b, Wd, bd,
        s_steps=S, has_bias=has_bias, has_bd=has_bd,
    )
    res = run_bass_kernel_spmd(nc, in_maps, core_ids=list(range(NCORES)))
    y = np.concatenate([res.results[k]["y"] for k in range(NCORES)], axis=0)
    return y.astype(np.float32)
